# revision 12
# baseline (speedup 1.0000x reference)
"""Distributed Trainium2 Bass kernel for the EnhancedGNN problem.

Strategy (8 NeuronCores, SPMD single program):
  - Nodes are partitioned into 8 contiguous ranges (3125 each, padded to 3200).
  - Each edge is assigned to the core owning its dst; edges are sorted by dst
    and padded per 128-node dst-block to a uniform tile count (baked at build).
  - Segment ops become one-hot matmuls on the PE; one-hot S is built on-chip
    via is_equal against an iota.
  - Dense per-node phases run in feature-transposed layout so BatchNorm
    affine+relu is a single per-partition ACT op; BN statistics are combined
    with small AllReduces; h1^T is AllGathered so every core can compute the
    full xl table for GAT gathers.
  - GATv2 softmax skips segment_max (mathematically equivalent; logits are
    O(1) bounded) and folds the division into the aggregated num/den.
  - All matmul operands are float32r (single-pass PE, 4x fp32 throughput at
    moving-dim >= 256); producers tag outputs f32r so walrus accepts them.
"""
import sys
import math
import numpy as np

sys.path.insert(0, '/opt/trn_rl_repo')

C = 8
P = 128

LAST_RESULT = None  # BassKernelResults of the most recent run (for test harness)
TRACE = False
PHASES = 'F'
BSUB = 9
DUMP = 'h0T_own'


def kernel(**inputs):
    return _run(inputs)


def _ceil_to(a, m):
    return ((a + m - 1) // m) * m


def _split(total, maxw, align=P):
    out = []
    done = 0
    while done < total:
        w = min(maxw, total - done)
        out.append((done, w))
        done += w
    return out


def _sigmoid(x):
    return 1.0 / (1.0 + math.exp(-float(x)))


def _prep_edges(psrc, pdst, ea_rows, n_blocks_total, T=None):
    """Sort edges by padded dst, group per 128-node dst block, pad each block
    to T*128 edges. meta[...,0]=src padded id, meta[...,1]=bits of
    float(dst offset in block | -1); eaT last row is bias-enable."""
    E2 = psrc.shape[0]
    EDIM = ea_rows.shape[1]
    order = np.argsort(pdst, kind='stable')
    ssrc = psrc[order]
    sdst = pdst[order]
    sea = ea_rows[order]
    blk = sdst // P
    off = (sdst % P).astype(np.float32)
    counts = np.bincount(blk, minlength=n_blocks_total)
    if T is None:
        T = max(1, int(math.ceil(counts.max() / P)))
    W = T * P
    starts = np.zeros(n_blocks_total, np.int64)
    starts[1:] = np.cumsum(counts)[:-1]
    pos = np.arange(E2) - starts[blk]
    flat = blk * W + pos
    srcs = np.zeros(n_blocks_total * W, np.int32)
    offs = np.full(n_blocks_total * W, -1.0, np.float32)
    eaT = np.zeros((n_blocks_total * W, EDIM + 1), np.float32)
    srcs[flat] = ssrc.astype(np.int32)
    offs[flat] = off
    eaT[flat, :EDIM] = sea
    eaT[flat, EDIM] = 1.0
    meta = np.stack([srcs, offs.view(np.int32)], -1).reshape(n_blocks_total, W, 2)
    eaT = eaT.reshape(n_blocks_total, W, EDIM + 1).transpose(0, 2, 1).copy()
    offs = offs.reshape(n_blocks_total, W)
    return T, meta, eaT, offs


def _run(inputs):
    from concourse.bass_utils import run_bass_kernel_spmd
    global LAST_RESULT
    nc, in_maps, OWN = _prep(inputs)
    res = run_bass_kernel_spmd(nc, in_maps, list(range(C)), trace=TRACE)
    LAST_RESULT = res
    out = np.concatenate(
        [res.results[c]['outT'][:, :OWN].T for c in range(C)], 0)
    return np.ascontiguousarray(out.astype(np.float32))


def _prep(inputs):
    from concourse import bass, bacc, mybir, tile

    x = np.asarray(inputs['x'], np.float32)
    ei = np.asarray(inputs['edge_index']).astype(np.int64)
    ea = np.asarray(inputs['edge_attr'], np.float32)
    N, IN_DIM = x.shape
    E = ei.shape[1]
    ip_w = np.asarray(inputs['ip_w'], np.float32)
    H = ip_w.shape[1]
    gat_wl = np.asarray(inputs['gat_wl'], np.float32)
    HH = gat_wl.shape[1]
    HEADS = HH // H
    EDIM = ea.shape[1]
    assert N % C == 0
    OWN = N // C
    NOP = _ceil_to(OWN, P)
    NB = NOP // P
    NPAD = C * NOP

    s0 = _sigmoid(np.asarray(inputs['skip0']))
    s1 = _sigmoid(np.asarray(inputs['skip1']))

    # ---- CPU graph prep ----
    src, dst = ei[0], ei[1]
    pad_id = lambda g: (g // OWN) * NOP + (g % OWN)
    psrc = pad_id(src)
    pdst = pad_id(dst)
    Tg, gine_meta, gine_eaT, _ = _prep_edges(psrc, pdst, ea, C * NB)
    loop = np.arange(N, dtype=np.int64)
    ploop = pad_id(loop)
    ea_mean = ea.mean(0)
    psrc2 = np.concatenate([psrc, ploop])
    pdst2 = np.concatenate([pdst, ploop])
    ea2 = np.concatenate([ea, np.broadcast_to(ea_mean, (N, EDIM))], 0)
    Ta, gat_meta, gat_eaT, gat_offs = _prep_edges(psrc2, pdst2, ea2, C * NB)

    # ---- weights prep ----
    ip_rhs = np.concatenate([ip_w, np.asarray(inputs['ip_b'], np.float32)[None, :]], 0)
    gine_e_rhs = np.concatenate([np.asarray(inputs['gine_edge_w'], np.float32),
                                 np.asarray(inputs['gine_edge_b'], np.float32)[None, :]], 0)
    w1 = np.asarray(inputs['gine_w1'], np.float32)
    w2 = np.asarray(inputs['gine_w2'], np.float32)
    w2a, w2b = w2[:H].copy(), w2[H:].copy()
    wr = np.asarray(inputs['gat_wr'], np.float32)
    blbr = np.asarray(inputs['gat_bl'], np.float32) + np.asarray(inputs['gat_br'], np.float32)
    we7 = np.concatenate([np.asarray(inputs['gat_we'], np.float32), blbr[None, :]], 0)
    attT = np.asarray(inputs['gat_att'], np.float32).T.copy()

    b1 = np.asarray(inputs['gine_b1'], np.float32)
    g1 = np.asarray(inputs['gine_gamma'], np.float32)
    be1 = np.asarray(inputs['gine_beta'], np.float32)
    colpack = np.stack([
        np.asarray(inputs['ip_gamma'], np.float32),               # 0
        np.asarray(inputs['ip_beta'], np.float32),                # 1
        b1[:H], b1[H:],                                           # 2,3
        g1[:H], g1[H:],                                           # 4,5
        be1[:H], be1[H:],                                         # 6,7
        (1.0 - s0) * np.asarray(inputs['gine_b2'], np.float32),   # 8
        np.asarray(inputs['norm0_gamma'], np.float32),            # 9
        np.asarray(inputs['norm0_beta'], np.float32),             # 10
        np.asarray(inputs['norm1_gamma'], np.float32) / 3.0,      # 11
        np.asarray(inputs['norm1_beta'], np.float32) / 3.0,       # 12
        (1.0 - s1) * np.asarray(inputs['gat_bias'], np.float32),  # 13
    ], 1)

    xT = np.zeros((IN_DIM + 1, NPAD), np.float32)
    for c in range(C):
        xs = x[c * OWN:(c + 1) * OWN]
        xT[:IN_DIM, c * NOP:c * NOP + OWN] = xs.T
        xT[IN_DIM, c * NOP:c * NOP + OWN] = 1.0
    iota = np.arange(P, dtype=np.float32)[:, None]
    identity = np.eye(P, dtype=np.float32)

    nc = _build(bass, bacc, mybir, tile,
                N=N, IN_DIM=IN_DIM, H=H, HEADS=HEADS, HH=HH, EDIM=EDIM,
                OWN=OWN, NOP=NOP, NB=NB, NPAD=NPAD, Tg=Tg, Ta=Ta,
                s0=s0, s1=s1)

    in_maps = []
    for c in range(C):
        own_rows = (c * NOP + np.arange(NOP, dtype=np.int32))[:, None]
        in_maps.append({
            'xT': xT, 'iota': iota, 'identity': identity,
            'own_rows': own_rows, 'colpack': colpack,
            'ip_rhs': ip_rhs, 'gine_e_rhs': gine_e_rhs, 'w1': w1,
            'w2a': w2a, 'w2b': w2b, 'wl': gat_wl, 'wr': wr, 'we7': we7,
            'attT': attT,
            'gine_meta': gine_meta[c * NB:(c + 1) * NB],
            'gine_eaT': gine_eaT[c * NB:(c + 1) * NB],
            'gat_meta': gat_meta[c * NB:(c + 1) * NB],
            'gat_eaT': gat_eaT[c * NB:(c + 1) * NB],
            'gat_dstrow': gat_offs[c * NB:(c + 1) * NB],
        })
    return nc, in_maps, OWN


def _build(bass, bacc, mybir, tile, *, N, IN_DIM, H, HEADS, HH,
           EDIM, OWN, NOP, NB, NPAD, Tg, Ta, s0, s1):
    F32 = mybir.dt.float32
    F32R = mybir.dt.float32r
    I32 = mybir.dt.int32
    AF = mybir.ActivationFunctionType
    OP = mybir.AluOpType
    AX = mybir.AxisListType
    inv_n = 1.0 / N
    EPS = 1e-5
    grp = [list(range(C))]

    nc = bacc.Bacc("TRN2", target_bir_lowering=False, debug=False)

    # ---- I/O ----
    xT_d = nc.dram_tensor('xT', [IN_DIM + 1, NPAD], F32R, kind='ExternalInput')
    iota_d = nc.dram_tensor('iota', [P, 1], F32, kind='ExternalInput')
    ident_d = nc.dram_tensor('identity', [P, P], F32R, kind='ExternalInput')
    own_d = nc.dram_tensor('own_rows', [NOP, 1], I32, kind='ExternalInput')
    colpack_d = nc.dram_tensor('colpack', [H, 14], F32, kind='ExternalInput')
    ip_rhs_d = nc.dram_tensor('ip_rhs', [IN_DIM + 1, H], F32R, kind='ExternalInput')
    gine_e_rhs_d = nc.dram_tensor('gine_e_rhs', [EDIM + 1, H], F32R, kind='ExternalInput')
    w1_d = nc.dram_tensor('w1', [H, 2 * H], F32R, kind='ExternalInput')
    w2a_d = nc.dram_tensor('w2a', [H, H], F32R, kind='ExternalInput')
    w2b_d = nc.dram_tensor('w2b', [H, H], F32R, kind='ExternalInput')
    wl_d = nc.dram_tensor('wl', [H, HH], F32R, kind='ExternalInput')
    wr_d = nc.dram_tensor('wr', [H, HH], F32R, kind='ExternalInput')
    we7_d = nc.dram_tensor('we7', [EDIM + 1, HH], F32R, kind='ExternalInput')
    attT_d = nc.dram_tensor('attT', [H, HEADS], F32, kind='ExternalInput')
    gmeta_d = nc.dram_tensor('gine_meta', [NB, Tg * P, 2], I32, kind='ExternalInput')
    geaT_d = nc.dram_tensor('gine_eaT', [NB, EDIM + 1, Tg * P], F32R, kind='ExternalInput')
    ameta_d = nc.dram_tensor('gat_meta', [NB, Ta * P, 2], I32, kind='ExternalInput')
    aeaT_d = nc.dram_tensor('gat_eaT', [NB, EDIM + 1, Ta * P], F32R, kind='ExternalInput')
    adst_d = nc.dram_tensor('gat_dstrow', [NB, Ta * P], F32, kind='ExternalInput')
    outT_d = nc.dram_tensor('outT', [H, NOP], F32, kind='ExternalOutput')

    chunks = _split(NOP, 640)
    NCH = C * len(chunks)

    with tile.TileContext(nc) as tc:
      with tc.tile_pool(name='consts', bufs=1) as cst, \
           tc.tile_pool(name='persist', bufs=1) as per, \
           tc.tile_pool(name='dram', bufs=1, space='DRAM') as dr:
        ident = cst.tile([P, P], F32R)
        nc.sync.dma_start(ident[:], ident_d[:])
        iota_col = cst.tile([P, 1], F32)
        nc.sync.dma_start(iota_col[:], iota_d[:])
        iota_rep = cst.tile([P, P], F32)
        nc.sync.dma_start(iota_rep[:], iota_d[:, 0][None, :].to_broadcast([P, P]))
        colpack_s = cst.tile([H, 14], F32)
        nc.sync.dma_start(colpack_s[:], colpack_d[:])
        ip_rhs_s = cst.tile([IN_DIM + 1, H], F32R)
        nc.sync.dma_start(ip_rhs_s[:], ip_rhs_d[:])
        ge_rhs_s = cst.tile([EDIM + 1, H], F32R)
        nc.sync.dma_start(ge_rhs_s[:], gine_e_rhs_d[:])
        w1_s = cst.tile([H, 2 * H], F32R)
        nc.sync.dma_start(w1_s[:], w1_d[:])
        w2a_s = cst.tile([H, H], F32R)
        nc.sync.dma_start(w2a_s[:], w2a_d[:])
        w2b_s = cst.tile([H, H], F32R)
        nc.sync.dma_start(w2b_s[:], w2b_d[:])
        wl_s = cst.tile([H, HH], F32R)
        nc.sync.dma_start(wl_s[:], wl_d[:])
        wr_s = cst.tile([H, HH], F32R)
        nc.sync.dma_start(wr_s[:], wr_d[:])
        we7_s = cst.tile([EDIM + 1, HH], F32R)
        nc.sync.dma_start(we7_s[:], we7_d[:])
        attT_s = cst.tile([H, HEADS], F32)
        nc.sync.dma_start(attT_s[:], attT_d[:])
        own_s = cst.tile([P, NB], I32)
        nc.sync.dma_start(own_s[:], own_d[:, 0].rearrange('(b p) -> p b', p=P))

        # ---- persistent SBUF state ----
        h0T_own = per.tile([P, NOP], F32)
        uaT = per.tile([P, NOP], F32)
        ubT = per.tile([P, NOP], F32)
        h1p = per.tile([P, NOP], F32)
        h1T_own = per.tile([P, NOP], F32R)
        h2p = per.tile([P, NOP], F32)
        scratch = per.tile([P, NOP], F32)
        xsum = per.tile([P, NCH], F32)
        x2sum = per.tile([P, NCH], F32)
        st1 = per.tile([P, 4], F32)
        st2 = per.tile([P, 2], F32)
        st3 = per.tile([P, 2], F32)
        vec = per.tile([P, 24], F32)

        # ---- DRAM internals ----
        h0_dram = dr.tile([NPAD, H], F32R)
        xl_dram = dr.tile([NPAD, HH], F32R)
        xr_dram = dr.tile([NOP, HH], F32R)
        h1T_own_dr = dr.tile([H, NOP], F32R)
        h1T_all = dr.tile([C, H, NOP], F32R, addr_space='Shared')
        st1_in = dr.tile([H, 4], F32)
        st1_out = dr.tile([H, 4], F32, addr_space='Shared')
        st2_in = dr.tile([H, 2], F32)
        st2_out = dr.tile([H, 2], F32, addr_space='Shared')
        st3_in = dr.tile([H, 2], F32)
        st3_out = dr.tile([H, 2], F32, addr_space='Shared')

        def affine_from_stats(stats_s, col0, gamma_col, beta_col, vc):
            mean = vec[:, vc + 2:vc + 3]
            ex2 = vec[:, vc + 3:vc + 4]
            var = vec[:, vc + 4:vc + 5]
            sd = vec[:, vc + 5:vc + 6]
            nc.scalar.mul(mean, stats_s[:, col0:col0 + 1], inv_n)
            nc.scalar.mul(ex2, stats_s[:, col0 + 1:col0 + 2], inv_n)
            nc.vector.tensor_tensor(out=var, in0=mean, in1=mean, op=OP.mult)
            nc.vector.tensor_tensor(out=var, in0=ex2, in1=var, op=OP.subtract)
            nc.vector.tensor_scalar_add(var, var, EPS)
            nc.scalar.activation(sd, var, AF.Sqrt)
            nc.vector.reciprocal(vec[:, vc:vc + 1], sd)
            nc.vector.tensor_tensor(out=vec[:, vc:vc + 1], in0=gamma_col,
                                    in1=vec[:, vc:vc + 1], op=OP.mult)
            nc.vector.tensor_tensor(out=sd, in0=mean, in1=vec[:, vc:vc + 1], op=OP.mult)
            nc.vector.tensor_tensor(out=vec[:, vc + 1:vc + 2], in0=beta_col,
                                    in1=sd, op=OP.subtract)

        # ================= Phase A: h0 =================
        with nc.named_scope('phA'), \
             tc.tile_pool(name='sbA', bufs=2) as sa, \
             tc.tile_pool(name='psA', bufs=2, space='PSUM') as pa:
            for a2 in (False, True):
                for o in range(C):
                    for (k0, w) in chunks:
                        gc = o * NOP + k0
                        ich = o * len(chunks) + chunks.index((k0, w))
                        xT_s = sa.tile([IN_DIM + 1, 640], F32R, tag='xT')
                        nc.sync.dma_start(xT_s[:, :w], xT_d[:, gc:gc + w])
                        ps_xw = pa.tile([P, 640], F32, tag='xw')
                        for (p0, pw) in _split(w, 512):
                            nc.tensor.matmul(ps_xw[:, p0:p0 + pw], lhsT=ip_rhs_s[:],
                                             rhs=xT_s[:, p0:p0 + pw], start=True, stop=True)
                        if not a2:
                            scrA = sa.tile([P, 640], F32, tag='scrA')
                            nc.scalar.activation(scrA[:, :w], ps_xw[:, :w], AF.Identity,
                                                 accum_out=xsum[:, ich:ich + 1])
                            scrB = sa.tile([P, 640], F32, tag='scrB')
                            nc.scalar.activation(scrB[:, :w], ps_xw[:, :w], AF.Square,
                                                 accum_out=x2sum[:, ich:ich + 1])
                        else:
                            h0T_s = sa.tile([P, 640], F32R, tag='h0T')
                            nc.scalar.activation(h0T_s[:, :w], ps_xw[:, :w], AF.Relu,
                                                 bias=vec[:, 1:2], scale=vec[:, 0:1])
                            ps_tr = pa.tile([P, 640], F32, tag='tr')
                            for t in range(w // P):
                                nc.tensor.matmul(ps_tr[:, t * P:(t + 1) * P],
                                                 lhsT=h0T_s[:, t * P:(t + 1) * P].bitcast(F32),
                                                 rhs=ident[:].bitcast(F32),
                                                 is_transpose=True,
                                                 start=True, stop=True)
                            h0r_s = sa.tile([P, 640], F32R, tag='h0r')
                            nc.vector.tensor_copy(h0r_s[:, :w], ps_tr[:, :w])
                            nc.sync.dma_start(
                                h0_dram[gc:gc + w, :].rearrange('(t p) f -> p t f', p=P),
                                h0r_s[:, :w].rearrange('p (t f) -> p t f', f=P))
                if not a2:
                    nc.vector.tensor_reduce(out=vec[:, 8:9], in_=xsum[:], axis=AX.X, op=OP.add)
                    nc.vector.tensor_reduce(out=vec[:, 9:10], in_=x2sum[:], axis=AX.X, op=OP.add)
                    affine_from_stats(vec, 8, colpack_s[:, 0:1], colpack_s[:, 1:2], 0)

        # ================= Phase B: GINE edges + u =================
        with nc.named_scope('phB'), \
             tc.tile_pool(name='sbB', bufs=2) as sb, \
             tc.tile_pool(name='psB', bufs=2, space='PSUM') as pb:
            # own h0 blocks -> transposed persist
            for b0 in range(0, NB if PHASES >= 'B' and BSUB >= 1 else 0, 4):
                bw = min(4, NB - b0)
                h0blk = sb.tile([P, 4, H], F32R, tag='h0blk')
                for i in range(bw):
                    nc.gpsimd.indirect_dma_start(
                        out=h0blk[:, i, :], out_offset=None, in_=h0_dram[:],
                        in_offset=bass.IndirectOffsetOnAxis(
                            ap=own_s[:, b0 + i:b0 + i + 1], axis=0))
                ps_trB = pb.tile([P, 4 * P], F32, tag='trB')
                for i in range(bw):
                    nc.tensor.matmul(ps_trB[:, i * P:(i + 1) * P],
                                     lhsT=h0blk[:, i, :].bitcast(F32),
                                     rhs=ident[:].bitcast(F32),
                                     is_transpose=True, start=True, stop=True)
                nc.vector.tensor_copy(h0T_own[:, b0 * P:(b0 + bw) * P], ps_trB[:, :bw * P])

            ngrp = (NB + 3) // 4
            for gb in range(ngrp if PHASES >= 'B' and BSUB >= 2 else 0):
                lanes = min(4, NB - gb * 4)
                ps_agg = pb.tile([P, 512], F32, tag='agg')
                for lane in range(lanes):
                    b = gb * 4 + lane
                    meta_s = sb.tile([P, Tg, 2], I32, tag='gmeta')
                    nc.sync.dma_start(
                        meta_s[:], gmeta_d[b].rearrange('(t p) k -> p t k', p=P))
                    eaT_s = sb.tile([EDIM + 1, Tg * P], F32R, tag='geaT')
                    nc.sync.dma_start(eaT_s[:], geaT_d[b])
                    for g4 in range(0, Tg, 4):
                        gw = min(4, Tg - g4)
                        S4 = sb.tile([P, 4, P], F32R, tag='S4g')
                        nc.vector.tensor_tensor(
                            out=S4[:, :gw, :],
                            in0=iota_rep[:].rearrange('p (o n) -> p o n', o=1
                                                      ).to_broadcast([P, gw, P]),
                            in1=meta_s[:, g4:g4 + gw, 1:2].bitcast(F32
                                                                   ).to_broadcast([P, gw, P]),
                            op=OP.is_equal)
                        hsrc = sb.tile([P, 4, H], F32R, tag='hsrc')
                        for i in range(gw):
                            nc.gpsimd.indirect_dma_start(
                                out=hsrc[:, i, :], out_offset=None,
                                in_=h0_dram[:],
                                in_offset=bass.IndirectOffsetOnAxis(
                                    ap=meta_s[:, g4 + i, 0:1], axis=0))
                        ps_e = pb.tile([P, 4 * H], F32, tag='e4')
                        for i in range(gw):
                            t = g4 + i
                            nc.tensor.matmul(ps_e[:, i * H:(i + 1) * H],
                                             lhsT=eaT_s[:, t * P:(t + 1) * P],
                                             rhs=ge_rhs_s[:], start=True, stop=True)
                        msg = sb.tile([P, 4 * H], F32R, tag='msg')
                        nc.vector.tensor_tensor(
                            out=msg[:, :gw * H],
                            in0=hsrc[:, :gw, :].rearrange('p a b -> p (a b)'),
                            in1=ps_e[:, :gw * H], op=OP.add)
                        nc.scalar.activation(msg[:, :gw * H], msg[:, :gw * H], AF.Relu)
                        for i in range(gw):
                            t = g4 + i
                            nc.tensor.matmul(ps_agg[:, lane * P:(lane + 1) * P],
                                             lhsT=msg[:, i * H:(i + 1) * H],
                                             rhs=S4[:, i, :],
                                             start=(t == 0), stop=(t == Tg - 1))
                cw = lanes * P
                c0 = gb * 4 * P
                zT = sb.tile([P, 512], F32R, tag='zT')
                nc.vector.tensor_tensor(out=zT[:, :cw], in0=h0T_own[:, c0:c0 + cw],
                                        in1=ps_agg[:, :cw], op=OP.add)
                ps_u = pb.tile([P, 512], F32, tag='u')
                nc.tensor.matmul(ps_u[:, :cw], lhsT=w1_s[:, :H], rhs=zT[:, :cw],
                                 start=True, stop=True)
                nc.scalar.activation(uaT[:, c0:c0 + cw], ps_u[:, :cw], AF.Identity,
                                     bias=colpack_s[:, 2:3])
                ps_u2 = pb.tile([P, 512], F32, tag='u')
                nc.tensor.matmul(ps_u2[:, :cw], lhsT=w1_s[:, H:2 * H], rhs=zT[:, :cw],
                                 start=True, stop=True)
                nc.scalar.activation(ubT[:, c0:c0 + cw], ps_u2[:, :cw], AF.Identity,
                                     bias=colpack_s[:, 3:4])

            # ---- stats #1 + AllReduce ----
            if PHASES >= 'B' and BSUB >= 3:
              _s = nc.enter_named_scope('st1', False)[0]
              nc.vector.tensor_reduce(out=st1[:, 0:1], in_=uaT[:, :OWN], axis=AX.X, op=OP.add)
              nc.scalar.activation(scratch[:, :OWN], uaT[:, :OWN], AF.Square,
                                   accum_out=st1[:, 1:2])
              nc.vector.tensor_reduce(out=st1[:, 2:3], in_=ubT[:, :OWN], axis=AX.X, op=OP.add)
              nc.scalar.activation(scratch[:, :OWN], ubT[:, :OWN], AF.Square,
                                   accum_out=st1[:, 3:4])
              nc.sync.dma_start(st1_in[:], st1[:])
              nc.gpsimd.collective_compute('AllReduce', mybir.AluOpType.add,
                                           replica_groups=grp,
                                           ins=[st1_in.opt()], outs=[st1_out.opt()])
              st1r = sb.tile([P, 4], F32, tag='st1r')
              nc.sync.dma_start(st1r[:], st1_out[:])
              affine_from_stats(st1r, 0, colpack_s[:, 4:5], colpack_s[:, 6:7], 0)
              affine_from_stats(st1r, 2, colpack_s[:, 5:6], colpack_s[:, 7:8], 8)
              nc.leave_named_scope('st1', _s, False)

            # ---- phase C ----
            _sc = nc.enter_named_scope('phC', False)[0]
            for gb in range(ngrp if PHASES >= 'C' else 0):
                lanes = min(4, NB - gb * 4)
                cw = lanes * P
                c0 = gb * 4 * P
                ua4 = sb.tile([P, 512], F32R, tag='ua4')
                nc.scalar.activation(ua4[:, :cw], uaT[:, c0:c0 + cw], AF.Relu,
                                     bias=vec[:, 1:2], scale=vec[:, 0:1])
                ub4 = sb.tile([P, 512], F32R, tag='ub4')
                nc.scalar.activation(ub4[:, :cw], ubT[:, c0:c0 + cw], AF.Relu,
                                     bias=vec[:, 9:10], scale=vec[:, 8:9])
                ps_y = pb.tile([P, 512], F32, tag='u')
                nc.tensor.matmul(ps_y[:, :cw], lhsT=w2a_s[:], rhs=ua4[:, :cw],
                                 start=True, stop=False)
                nc.tensor.matmul(ps_y[:, :cw], lhsT=w2b_s[:], rhs=ub4[:, :cw],
                                 start=False, stop=True)
                relug = sb.tile([P, 512], F32, tag='relug')
                nc.scalar.activation(relug[:, :cw], ps_y[:, :cw], AF.Relu,
                                     bias=colpack_s[:, 8:9], scale=(1.0 - s0))
                th0 = sb.tile([P, 512], F32, tag='th0')
                nc.vector.tensor_scalar_mul(th0[:, :cw], h0T_own[:, c0:c0 + cw], s0)
                nc.vector.tensor_tensor(out=h1p[:, c0:c0 + cw], in0=th0[:, :cw],
                                        in1=relug[:, :cw], op=OP.add)
            if PHASES >= 'C':
              nc.vector.tensor_reduce(out=st2[:, 0:1], in_=h1p[:, :OWN], axis=AX.X, op=OP.add)
              nc.scalar.activation(scratch[:, :OWN], h1p[:, :OWN], AF.Square,
                                   accum_out=st2[:, 1:2])
              nc.sync.dma_start(st2_in[:], st2[:])
              nc.gpsimd.collective_compute('AllReduce', mybir.AluOpType.add,
                                           replica_groups=grp,
                                           ins=[st2_in.opt()], outs=[st2_out.opt()])
              st2r = sb.tile([P, 2], F32, tag='st2r')
              nc.sync.dma_start(st2r[:], st2_out[:])
              affine_from_stats(st2r, 0, colpack_s[:, 9:10], colpack_s[:, 10:11], 0)
              for gb in range(ngrp):
                  lanes = min(4, NB - gb * 4)
                  cw = lanes * P
                  c0 = gb * 4 * P
                  nc.scalar.activation(h1T_own[:, c0:c0 + cw], h1p[:, c0:c0 + cw],
                                       AF.Identity, bias=vec[:, 1:2], scale=vec[:, 0:1])
              nc.sync.dma_start(h1T_own_dr[:], h1T_own[:])
              nc.gpsimd.collective_compute('AllGather', mybir.AluOpType.bypass,
                                           replica_groups=grp,
                                           ins=[h1T_own_dr.opt()], outs=[h1T_all.opt()])
            nc.leave_named_scope('phC', _sc, False)

        # ================= Phase D: xl / xr tables =================
        with nc.named_scope('phD'), \
             tc.tile_pool(name='sbD', bufs=3) as sd_, \
             tc.tile_pool(name='psD', bufs=2, space='PSUM') as pd_:
            for j in range(C * NB if PHASES >= 'D' else 0):
                h1blk = sd_.tile([P, P], F32R, tag='h1blk')
                nc.sync.dma_start(h1blk[:],
                                  h1T_all[j // NB, :, (j % NB) * P:(j % NB + 1) * P])
                ps_xl = pd_.tile([P, HH], F32, tag='xl')
                nc.tensor.matmul(ps_xl[:], lhsT=h1blk[:], rhs=wl_s[:], start=True, stop=True)
                xl_s = sd_.tile([P, HH], F32R, tag='xls')
                if j % 2 == 0:
                    nc.scalar.copy(xl_s[:], ps_xl[:])
                else:
                    nc.vector.tensor_copy(xl_s[:], ps_xl[:])
                nc.sync.dma_start(xl_dram[j * P:(j + 1) * P, :], xl_s[:])
            for b in range(NB if PHASES >= 'D' else 0):
                ps_xr = pd_.tile([P, HH], F32, tag='xl')
                nc.tensor.matmul(ps_xr[:], lhsT=h1T_own[:, b * P:(b + 1) * P],
                                 rhs=wr_s[:], start=True, stop=True)
                xr_c = sd_.tile([P, HH], F32R, tag='xls')
                nc.vector.tensor_copy(xr_c[:], ps_xr[:])
                nc.sync.dma_start(xr_dram[b * P:(b + 1) * P, :], xr_c[:])

        # ================= Phase E: GAT =================
        # psum_v layout per 2-tile group: col = h*(g2*128) + t*128 + e
        with nc.named_scope('phE'), \
             tc.tile_pool(name='sbE', bufs=2) as se, \
             tc.tile_pool(name='psE', bufs=1, space='PSUM') as pe:
            for b in range(NB if PHASES >= 'E' else 0):
                metg = se.tile([P, Ta, 2], I32, tag='ameta')
                nc.sync.dma_start(metg[:], ameta_d[b].rearrange('(t p) k -> p t k', p=P))
                eaTg = se.tile([EDIM + 1, Ta * P], F32R, tag='aeaT')
                nc.sync.dma_start(eaTg[:], aeaT_d[b])
                xr_s = se.tile([P, HH], F32R, tag='xr')
                nc.sync.dma_start(xr_s[:], xr_dram[b * P:(b + 1) * P, :])
                ps_num = pe.tile([P, HEADS * P], F32, tag='num', bufs=1)
                ps_den = pe.tile([P, HEADS], F32, tag='den', bufs=1)
                for t2 in range(0, Ta, 2):
                    g2 = min(2, Ta - t2)
                    gg = g2 * P
                    ps_v = pe.tile([P, 2 * HH], F32, tag='vT', bufs=2)
                    xl_g = se.tile([P, 2, HH], F32R, tag='xlg', bufs=3)
                    for i in range(g2):
                        nc.gpsimd.indirect_dma_start(
                            out=xl_g[:, i, :], out_offset=None,
                            in_=xl_dram[:],
                            in_offset=bass.IndirectOffsetOnAxis(
                                ap=metg[:, t2 + i, 0:1], axis=0))
                    S2 = se.tile([P, 2, P], F32R, tag='S2', bufs=2)
                    nc.vector.tensor_tensor(
                        out=S2[:, :g2, :],
                        in0=iota_rep[:].rearrange('p (o n) -> p o n', o=1
                                                  ).to_broadcast([P, g2, P]),
                        in1=metg[:, t2:t2 + g2, 1:2].bitcast(F32
                                                             ).to_broadcast([P, g2, P]),
                        op=OP.is_equal)
                    dstrep = se.tile([P, 2 * P], F32, tag='dstrep', bufs=2)
                    nc.sync.dma_start(
                        dstrep[:, :gg],
                        adst_d[b, t2 * P:(t2 + g2) * P][None, :].to_broadcast([P, gg]))
                    ST2 = se.tile([P, 2 * P], F32R, tag='ST2', bufs=2)
                    nc.vector.tensor_scalar(
                        out=ST2[:, :gg], in0=dstrep[:, :gg], scalar1=iota_col[:, 0:1],
                        scalar2=None, op0=OP.is_equal)
                    for h in range(HEADS):
                        hc = h * gg
                        nc.tensor.matmul(ps_v[:, hc:hc + gg],
                                         lhsT=we7_s[:, h * H:(h + 1) * H],
                                         rhs=eaTg[:, t2 * P:(t2 + g2) * P],
                                         start=True, stop=False)
                        nc.tensor.matmul(ps_v[:, hc:hc + gg],
                                         lhsT=xr_s[:, h * H:(h + 1) * H],
                                         rhs=ST2[:, :gg], start=False, stop=False)
                        for i in range(g2):
                            nc.tensor.matmul(
                                ps_v[:, hc + i * P:hc + (i + 1) * P],
                                lhsT=xl_g[:, i, h * H:(h + 1) * H].bitcast(F32),
                                rhs=ident[:].bitcast(F32), is_transpose=True,
                                start=False, stop=(i == g2 - 1))
                    mT = se.tile([P, 2 * HH], F32, tag='mT', bufs=2)
                    nc.scalar.activation(mT[:, :g2 * HH], ps_v[:, :g2 * HH],
                                         AF.Prelu, alpha=0.2)
                    ps_lg = pe.tile([P, 2 * HEADS], F32, tag='lg', bufs=1)
                    for i in range(g2):
                        for h in range(HEADS):
                            nc.tensor.matmul(
                                ps_lg[:, i * HEADS + h:i * HEADS + h + 1],
                                lhsT=mT[:, h * gg + i * P:h * gg + (i + 1) * P],
                                rhs=attT_s[:, h:h + 1], start=True, stop=True)
                    w_s = se.tile([P, 2 * HEADS], F32R, tag='wexp', bufs=2)
                    nc.scalar.activation(w_s[:, :g2 * HEADS], ps_lg[:, :g2 * HEADS], AF.Exp)
                    wxl = se.tile([P, 2 * HEADS, H], F32R, tag='wxl', bufs=2)
                    nc.vector.tensor_tensor(
                        out=wxl[:, :g2 * HEADS, :],
                        in0=xl_g[:, :g2, :].rearrange('p a (h f) -> p (a h) f', f=H),
                        in1=w_s[:, :g2 * HEADS].rearrange('p (e o) -> p e o', o=1
                                                          ).to_broadcast([P, g2 * HEADS, H]),
                        op=OP.mult)
                    for i in range(g2):
                        t = t2 + i
                        nc.tensor.matmul(
                            ps_num[:],
                            lhsT=S2[:, i, :],
                            rhs=wxl[:, i * HEADS:(i + 1) * HEADS, :].rearrange(
                                'p h f -> p (h f)'),
                            start=(t == 0), stop=(t == Ta - 1))
                        nc.tensor.matmul(
                            ps_den[:], lhsT=S2[:, i, :],
                            rhs=w_s[:, i * HEADS:(i + 1) * HEADS],
                            start=(t == 0), stop=(t == Ta - 1))
                # ---- block recombination (row-major num/den) ----
                den_s = se.tile([P, HEADS], F32, tag='dens', bufs=2)
                nc.scalar.copy(den_s[:], ps_den[:])
                r_s = se.tile([P, HEADS], F32, tag='rs', bufs=2)
                nc.vector.reciprocal(r_s[:], den_s[:])
                prod = se.tile([P, HEADS, P], F32, tag='prod', bufs=2)
                nc.vector.tensor_tensor(
                    out=prod[:],
                    in0=ps_num[:].rearrange('p (h n) -> p h n', h=HEADS),
                    in1=r_s[:].rearrange('p (h o) -> p h o', o=1
                                         ).to_broadcast([P, HEADS, P]),
                    op=OP.mult)
                gR = se.tile([P, P], F32, tag='gR', bufs=2)
                nc.vector.tensor_reduce(out=gR[:], in_=prod[:].rearrange('p h n -> p n h'),
                                        axis=AX.X, op=OP.add)
                ps_gT = pe.tile([P, P], F32, tag='lg', bufs=1)
                nc.tensor.matmul(ps_gT[:], lhsT=gR[:], rhs=ident[:].bitcast(F32),
                                 is_transpose=True, start=True, stop=True)
                relug2 = se.tile([P, P], F32, tag='relug2', bufs=2)
                nc.scalar.activation(relug2[:], ps_gT[:], AF.Relu,
                                     bias=colpack_s[:, 13:14], scale=(1.0 - s1) / 4.0)
                th1 = se.tile([P, P], F32, tag='th1', bufs=2)
                nc.vector.tensor_scalar_mul(th1[:], h1T_own[:, b * P:(b + 1) * P], s1)
                nc.vector.tensor_tensor(out=h2p[:, b * P:(b + 1) * P], in0=th1[:],
                                        in1=relug2[:], op=OP.add)

            # ---- stats #3 + AllReduce + final ----
            if PHASES >= 'F':
              _sf = nc.enter_named_scope('phF', False)[0]
              nc.vector.tensor_reduce(out=st3[:, 0:1], in_=h2p[:, :OWN], axis=AX.X, op=OP.add)
              nc.scalar.activation(scratch[:, :OWN], h2p[:, :OWN], AF.Square,
                                   accum_out=st3[:, 1:2])
              nc.sync.dma_start(st3_in[:], st3[:])
              nc.gpsimd.collective_compute('AllReduce', mybir.AluOpType.add,
                                           replica_groups=grp,
                                           ins=[st3_in.opt()], outs=[st3_out.opt()])
              st3r = se.tile([P, 2], F32, tag='st3r')
              nc.sync.dma_start(st3r[:], st3_out[:])
              affine_from_stats(st3r, 0, colpack_s[:, 11:12], colpack_s[:, 12:13], 0)

              for (k0, w) in chunks:
                  t1 = se.tile([P, 640], F32, tag='f1', bufs=2)
                  nc.vector.tensor_tensor(out=t1[:, :w], in0=h0T_own[:, k0:k0 + w],
                                          in1=h1T_own[:, k0:k0 + w], op=OP.add)
                  nc.vector.tensor_scalar_mul(t1[:, :w], t1[:, :w], 1.0 / 3.0)
                  o2 = se.tile([P, 640], F32, tag='f2', bufs=2)
                  nc.scalar.activation(o2[:, :w], h2p[:, k0:k0 + w], AF.Identity,
                                       bias=vec[:, 1:2], scale=vec[:, 0:1])
                  nc.vector.tensor_tensor(out=t1[:, :w], in0=t1[:, :w], in1=o2[:, :w],
                                          op=OP.add)
                  nc.sync.dma_start(outT_d[:, k0:k0 + w], t1[:, :w])
              nc.leave_named_scope('phF', _sf, False)
            if PHASES < 'F':
                named = {'h0T_own': h0T_own, 'uaT': uaT, 'ubT': ubT, 'h1p': h1p,
                         'h1T_own': h1T_own, 'h2p': h2p}
                for tt_ in (h0T_own, uaT, ubT, h1p, h1T_own, h2p, scratch,
                            xsum, x2sum, st1, st2, st3, vec):
                    nc.gpsimd.memset(tt_[:, 0:1], 0.0)
                nc.sync.dma_start(outT_d[:], named[DUMP][:])

    nc.finalize()
    return nc

